# revision 8
# baseline (speedup 1.0000x reference)
"""SSD NMS detection kernel for 8 Trainium2 NeuronCores.

Sharding: data-parallel over batch — one image per core (8 images, 8 cores).
As part of sharding, the host re-lays-out each image's conf tensor class-major
([A, C] -> [C_pad, A]) so each core can DMA per-class rows contiguously;
anchors/loc are per-image inputs. All compute (decode, top-k selection, exact
sort/merge, NMS, output masking) runs on-device.

Per-core pipeline:
  1. Decode all 65536 anchors to xyxy boxes (ACT exp + DVE), store to DRAM.
  2. gpsimd.topk (vocab 65408) per 8-class group -> exact sorted top-256
     (value, index) per class.  The 128-anchor remainder (topk vocab field is
     u16) is covered by vector.max8 top-8 + exact merge.
  3. Reverse to descending, stable-fix ties by index, merge remainder.
  4. One indirect DMA gathers the top-200 boxes per class.
  5. Greedy NMS: 199 sequential steps of fused custom-DVE ops over
     [81 classes, j] lanes; state encodes alive/area threshold.
  6. Mask suppressed entries, DMA out [81, 200, 5] per core.
"""

import numpy as np

import concourse.bass as bass
import concourse.bacc as bacc
import concourse.bass_isa as bass_isa
import concourse.mybir as mybir
import concourse.tile as tile
from concourse.bass_utils import run_bass_kernel_spmd

F32 = mybir.dt.float32
U32 = mybir.dt.uint32
ALU = mybir.AluOpType

B, A, C, K = 8, 65536, 81, 200
CP = 88          # classes padded to 11 groups of 8
NG = 11
V = 65408        # topk vocab (65536 overflows the u16 field); remainder = 128
REM = A - V
NMS_T = 0.45
CONF_T = 0.05
ALPHA = NMS_T / (1.0 + NMS_T)   # iou>t  <=>  inter > ALPHA*(area_i+area_j)
BIG = 1e30
HALF_BIG = 5e29
NCAND = 256      # candidates per class from topk
W = 264          # working width (256 + up to 8 remainder inserts)

_nc_cache = {}


def _register_custom_ops():
    from concourse.dve_ops import OPS, DveOp, CUSTOM_DVE_SPECS, has_src1
    from concourse.dve_spec import (
        Spec, Src0, Src1, C0, C1, Zero, relu, minn, maxx, select, lower,
    )
    from concourse.dve_uop import DveOpSpec

    if any(op.name == "NMS_OW" for op in OPS):
        from concourse.dve_ops import OPS as _O
        return {op.name: op for op in _O if op.name in ("NMS_OW", "NMS_UPD")}

    def mk(name, spec):
        shas = {}
        for ver in ("v3", "v4"):
            s = DveOpSpec(name=name, opcode=0, uops=lower(spec, ver=ver),
                          rd1_en=has_src1(spec))
            shas[ver] = s.sha(ver)
        op = DveOp(name, spec, subdim=False, uops_sha=shas)
        OPS.append(op)
        CUSTOM_DVE_SPECS[name] = spec
        import concourse.dve_ops as _dvo
        _dvo._SUB_OPCODE_FOR_NAME[name] = _dvo._CUSTOM_DVE_ROW_BASE + len(OPS) - 1
        assert _dvo._SUB_OPCODE_FOR_NAME[name] < 0x20
        return op

    # overlap extent: relu(min(hi_j, hi_i) - max(lo_j, lo_i))
    ow = mk("NMS_OW", Spec(
        body=relu(minn(Src0, C0) - maxx(Src1, C1)),
        reference=lambda in0, in1, s0, s1, imm2: np.maximum(
            np.minimum(in0, s0) - np.maximum(in1, s1), 0.0),
    ))
    # state update: state_j += (inter - alpha_i > state_j) ? big_k_i : 0
    upd = mk("NMS_UPD", Spec(
        body=Src1 + select((Src0 - C0) > Src1, C1, Zero),
        reference=lambda in0, in1, s0, s1, imm2: in1 + np.where(
            (in0 - s0) > in1, s1, 0.0),
    ))
    return {"NMS_OW": ow, "NMS_UPD": upd}


def _topk_raw(nc, out_ap, in_ap, tokens, vocab, k=256):
    g = nc.gpsimd
    return g.add_instruction(bass_isa.InstTopk(
        name=f"I-{nc.next_id()}",
        ins=[g.lower_ap(in_ap, for_isa=True)],
        outs=[g.lower_ap(out_ap, for_isa=True)],
        _tokens=tokens, _n=vocab, _k=k,
    ))


def _ap(t, off, dims):
    return bass.AP(t, off, [list(d) for d in dims])


def build_nc():
    if "nc" in _nc_cache:
        return _nc_cache["nc"]
    ops = _register_custom_ops()
    OW, UPD = ops["NMS_OW"], ops["NMS_UPD"]

    nc = bacc.Bacc("TRN2", target_bir_lowering=False, debug=False, num_devices=8)
    conf = nc.dram_tensor("conf_t", [CP, A], F32, kind="ExternalInput")
    loc = nc.dram_tensor("loc", [A, 4], F32, kind="ExternalInput")
    anch = nc.dram_tensor("anchors", [A, 4], F32, kind="ExternalInput")
    out = nc.dram_tensor("out", [C, 1000], F32, kind="ExternalOutput")
    boxtab = nc.dram_tensor("boxtab", [A, 4], F32, kind="Internal")

    sb = nc.alloc_sbuf_tensor
    # decode stage tiles: partition p holds anchors [512p, 512p+512)
    SL = sb("SL", [128, 2048], F32)
    SA = sb("SA", [128, 2048], F32)
    TE = sb("TE", [128, 1024], F32)   # exp(0.2*l_wh)
    WD = sb("WD", [128, 1024], F32)   # w,h decoded
    CX = sb("CX", [128, 1024], F32)   # cx,cy decoded
    BOXT = sb("BOXT", [128, 2048], F32)
    # topk
    TIN = [sb(f"TIN{i}", [128, 4088], F32) for i in range(2)]
    TK = sb("TK", [128, 32 * NG], U32)
    # remainder
    RMT = sb("RMT", [C, REM], F32)
    RV = sb("RV", [C, 8], F32)
    RIU = sb("RIU", [C, 8], U32)
    RI = sb("RI", [C, 8], F32)
    # candidate arrays (ping-pong)
    VA = sb("VA", [C, W], F32)
    IA = sb("IA", [C, W], F32)
    VB = sb("VB", [C, W], F32)
    IB = sb("IB", [C, W], F32)
    IAU = sb("IAU", [C, NCAND], U32)
    IDXU = sb("IDXU", [C, K], U32)
    # NMS
    BX = sb("BX", [C, K * 4], F32)
    AL = sb("AL", [C, K], F32)       # alpha_j = ALPHA*area_j
    ST = sb("ST", [C, K], F32)       # state
    OWT = sb("OWT", [C, K], F32)
    OHT = sb("OHT", [C, K], F32)
    RT = sb("RT", [C, K], F32)
    KB = sb("KB", [C, 1], F32)
    ALV = sb("ALV", [C, K], F32)
    OUTT = sb("OUTT", [C, 1000], F32)
    T0 = sb("T0", [C, W], F32)
    T1 = sb("T1", [C, W], F32)
    T2 = sb("T2", [C, W], F32)
    M0 = sb("M0", [C, W], mybir.dt.uint8)
    M1 = sb("M1", [C, W], mybir.dt.uint8)

    vec = nc.vector
    act = nc.scalar

    with tile.TileContext(nc) as tc:
        # ---- stage 1: decode boxes, write boxtab -------------------------
        nc.sync.dma_start(SL[:], _ap(loc, 0, [[2048, 128], [1, 2048]]))
        nc.sync.dma_start(SA[:], _ap(anch, 0, [[2048, 128], [1, 2048]]))
        sl_xy = _ap(SL, 0, [[2048, 128], [4, 512], [1, 2]])
        sl_wh = _ap(SL, 2, [[2048, 128], [4, 512], [1, 2]])
        sa_xy = _ap(SA, 0, [[2048, 128], [4, 512], [1, 2]])
        sa_wh = _ap(SA, 2, [[2048, 128], [4, 512], [1, 2]])
        te_v = _ap(TE, 0, [[1024, 128], [1, 1024]])
        wd_v = _ap(WD, 0, [[1024, 128], [1, 1024]])
        cx_v = _ap(CX, 0, [[1024, 128], [1, 1024]])
        # exp(l_wh*0.2)
        act.activation(te_v, sl_wh, mybir.ActivationFunctionType.Exp, scale=0.2)
        # wh = a_wh * exp(...)
        vec.tensor_mul(wd_v, sa_wh, te_v)
        # cxy = a_xy + (l_xy*0.1)*a_wh   (match reference eval order)
        vec.tensor_scalar_mul(cx_v, sl_xy, 0.1)
        vec.scalar_tensor_tensor(cx_v, cx_v, 1.0, sa_wh, op0=ALU.mult, op1=ALU.mult)
        vec.tensor_add(cx_v, cx_v, sa_xy)
        # box lo/hi = cxy -/+ wh*0.5
        box_lo = _ap(BOXT, 0, [[2048, 128], [4, 512], [1, 2]])
        box_hi = _ap(BOXT, 2, [[2048, 128], [4, 512], [1, 2]])
        vec.scalar_tensor_tensor(box_lo, wd_v, -0.5, cx_v, op0=ALU.mult, op1=ALU.add)
        vec.scalar_tensor_tensor(box_hi, wd_v, 0.5, cx_v, op0=ALU.mult, op1=ALU.add)
        nc.sync.dma_start(_ap(boxtab, 0, [[2048, 128], [1, 2048]]), BOXT[:])

        # ---- stage 2: topk per 8-class group -----------------------------
        for g in range(NG):
            t = TIN[g % 2]
            src = _ap(conf, 8 * g * A, [[A, 8], [4088, 16], [1, 4088]])
            nc.sync.dma_start(t[:], src)
            _topk_raw(nc, TK[:, 32 * g:32 * (g + 1)], t[:], tokens=8, vocab=V)

        # ---- stage 3: remainder top-8 ------------------------------------
        nc.sync.dma_start(RMT[:], _ap(conf, V, [[A, C], [1, REM]]))
        vec.max(out=RV[:], in_=RMT[:])
        vec.max_index(out=RIU[:], in_max=RV[:], in_values=RMT[:])
        # anchor = V + pos (kept in f32; indices < 2^24 are exact)
        vec.tensor_copy(RI[:], RIU[:])
        vec.tensor_scalar_add(RI[:], RI[:], float(V))

        # ---- stage 4: reorganize topk outs to [C, 256] asc ---------------
        for g in range(NG):
            ncls = 8 if g < 10 else 1   # classes 81..87 are padding
            src_v = _ap(TK, 32 * g, [[32 * NG, 16 * ncls], [1, 16]])
            dst_v = bass.AP(VA, 8 * g * W, [[W, ncls], [16, 16], [1, 16]])
            nc.sync.dma_start(dst_v, src_v.bitcast(F32))
            src_i = _ap(TK, 32 * g + 16, [[32 * NG, 16 * ncls], [1, 16]])
            dst_i = bass.AP(IAU, 8 * g * NCAND, [[NCAND, ncls], [16, 16], [1, 16]])
            nc.sync.dma_start(dst_i, src_i)
        # VA holds u32-bitcast floats; copy+reverse into VB/IB (desc order)
        va_rev = _ap(VA, NCAND - 1, [[W, C], [-1, NCAND]])
        iau_rev = _ap(IAU, NCAND - 1, [[NCAND, C], [-1, NCAND]])
        vec.tensor_copy(_ap(VB, 0, [[W, C], [1, NCAND]]), va_rev)
        vec.tensor_copy(_ap(IB, 0, [[W, C], [1, NCAND]]), iau_rev)
        vec.memset(_ap(VB, NCAND, [[W, C], [1, W - NCAND]]), 0.0)
        vec.memset(_ap(IB, NCAND, [[W, C], [1, W - NCAND]]), 0.0)

        # ---- stage 5: stable tie-fix (equal values -> ascending index) ---
        cur_v, cur_i, alt_v, alt_i = VB, IB, VA, IA
        for p in range(4):
            off = p % 2
            npair = (NCAND - off) // 2
            v_e = _ap(cur_v, off, [[W, C], [2, npair]])
            v_o = _ap(cur_v, off + 1, [[W, C], [2, npair]])
            i_e = _ap(cur_i, off, [[W, C], [2, npair]])
            i_o = _ap(cur_i, off + 1, [[W, C], [2, npair]])
            c_eq = _ap(M0, 0, [[W, C], [1, npair]])
            c_gt = _ap(M1, 0, [[W, C], [1, npair]])
            vec.tensor_tensor(c_eq, v_e, v_o, op=ALU.is_equal)
            vec.tensor_tensor(c_gt, i_e, i_o, op=ALU.is_gt)
            vec.tensor_tensor(c_eq, c_eq, c_gt, op=ALU.logical_and)
            # swap indices where c_eq
            ae = _ap(alt_i, off, [[W, C], [2, npair]])
            ao = _ap(alt_i, off + 1, [[W, C], [2, npair]])
            vec.select(ae, c_eq, i_o, i_e)
            vec.select(ao, c_eq, i_e, i_o)
            # copy the untouched boundary slots
            if off == 1:
                vec.tensor_copy(_ap(alt_i, 0, [[W, C], [1, 1]]),
                                _ap(cur_i, 0, [[W, C], [1, 1]]))
            if off + 2 * npair < W:
                n_tail = W - (off + 2 * npair)
                vec.tensor_copy(_ap(alt_i, off + 2 * npair, [[W, C], [1, n_tail]]),
                                _ap(cur_i, off + 2 * npair, [[W, C], [1, n_tail]]))
            cur_i, alt_i = alt_i, cur_i
        # values unchanged by tie-fix; cur = (cur_v=VB values, cur_i indices)

        # ---- stage 6: merge remainder top-8 ------------------------------
        # insert rv at its exact rank: out[p] = !b[p] ? cur[p]
        #                                       : (b[p-1] ? cur[p-1] : rv)
        # with b[p] = rv beats cur[p] (monotone 0..0 1..1), b[-1]=0, b[m]=1.
        for e in range(8):
            rv = RV[:, e:e + 1]
            ri = RI[:, e:e + 1]
            m = NCAND + e       # input width; output width m+1
            vsl = _ap(cur_v, 0, [[W, C], [1, m]])
            isl = _ap(cur_i, 0, [[W, C], [1, m]])
            b = _ap(M0, 0, [[W, C], [1, m]])
            t1 = _ap(M1, 0, [[W, C], [1, m]])
            t2 = _ap(T2, 0, [[W, C], [1, m]])
            # beats = (v < rv) | ((v == rv) & (i > ri))
            vec.tensor_scalar(b, vsl, rv, None, op0=ALU.is_lt)
            vec.tensor_scalar(t1, vsl, rv, None, op0=ALU.is_equal)
            vec.tensor_scalar(t2, isl, ri, None, op0=ALU.is_gt)
            vec.tensor_tensor(t1, t1, t2, op=ALU.logical_and)
            vec.tensor_tensor(b, b, t1, op=ALU.logical_or)
            # s1[p] = select(b[p-1], cur[p-1], rv) for p in 1..m  (width m)
            bm = _ap(M0, 0, [[W, C], [1, m]])
            rvb = _ap(RV, e, [[8, C], [0, m]])
            rib = _ap(RI, e, [[8, C], [0, m]])
            s1v = _ap(T1, 0, [[W, C], [1, m]])
            s2i = _ap(T2, 0, [[W, C], [1, m]])
            vec.select(s1v, bm, vsl, rvb)
            vec.select(s2i, bm, isl, rib)
            # out[p] = select(b[p], s1[p], cur[p]) for p in 1..m-1
            b1 = _ap(M0, 1, [[W, C], [1, m - 1]])
            vec.select(_ap(alt_v, 1, [[W, C], [1, m - 1]]), b1,
                       _ap(T1, 0, [[W, C], [1, m - 1]]),
                       _ap(cur_v, 1, [[W, C], [1, m - 1]]))
            vec.select(_ap(alt_i, 1, [[W, C], [1, m - 1]]), b1,
                       _ap(T2, 0, [[W, C], [1, m - 1]]),
                       _ap(cur_i, 1, [[W, C], [1, m - 1]]))
            # out[m] = s1[m] ; out[0] = select(b[0], rv, cur[0])
            vec.tensor_copy(_ap(alt_v, m, [[W, C], [1, 1]]),
                            _ap(T1, m - 1, [[W, C], [1, 1]]))
            vec.tensor_copy(_ap(alt_i, m, [[W, C], [1, 1]]),
                            _ap(T2, m - 1, [[W, C], [1, 1]]))
            vec.select(_ap(alt_v, 0, [[W, C], [1, 1]]), _ap(M0, 0, [[W, C], [1, 1]]),
                       rv, _ap(cur_v, 0, [[W, C], [1, 1]]))
            vec.select(_ap(alt_i, 0, [[W, C], [1, 1]]), _ap(M0, 0, [[W, C], [1, 1]]),
                       ri, _ap(cur_i, 0, [[W, C], [1, 1]]))
            cur_v, alt_v = alt_v, cur_v
            cur_i, alt_i = alt_i, cur_i

        # ---- stage 7: gather top-200 boxes -------------------------------
        vec.tensor_copy(IDXU[:], _ap(cur_i, 0, [[W, C], [1, K]]))
        idx200 = IDXU[:]
        nc.gpsimd.indirect_dma_start(
            out=BX[:], out_offset=None,
            in_=boxtab.ap(),
            in_offset=bass.IndirectOffsetOnAxis(ap=idx200, axis=0),
        )

        # ---- stage 8: alpha + state init ---------------------------------
        x1 = _ap(BX, 0, [[K * 4, C], [4, K]])
        y1 = _ap(BX, 1, [[K * 4, C], [4, K]])
        x2 = _ap(BX, 2, [[K * 4, C], [4, K]])
        y2 = _ap(BX, 3, [[K * 4, C], [4, K]])
        v200 = _ap(cur_v, 0, [[W, C], [1, K]])
        vec.tensor_sub(OWT[:], x2, x1)
        vec.tensor_sub(OHT[:], y2, y1)
        vec.tensor_mul(AL[:], OWT[:], OHT[:])
        vec.tensor_scalar_mul(AL[:], AL[:], ALPHA)
        # state = alpha + BIG*(v <= CONF_T)
        vec.tensor_scalar(ST[:], v200, CONF_T, BIG, op0=ALU.is_le, op1=ALU.mult)
        vec.tensor_add(ST[:], ST[:], AL[:])

        # ---- stage 9: greedy NMS, 199 steps ------------------------------
        for i in range(K - 1):
            w = K - 1 - i
            # kb = (state_i < HALF_BIG) * BIG
            vec.tensor_scalar(KB[:], ST[:, i:i + 1], HALF_BIG, BIG,
                              op0=ALU.is_lt, op1=ALU.mult)
            x2j = _ap(BX, 4 * (i + 1) + 2, [[K * 4, C], [4, w]])
            x1j = _ap(BX, 4 * (i + 1) + 0, [[K * 4, C], [4, w]])
            y2j = _ap(BX, 4 * (i + 1) + 3, [[K * 4, C], [4, w]])
            y1j = _ap(BX, 4 * (i + 1) + 1, [[K * 4, C], [4, w]])
            owv = _ap(OWT, 0, [[K, C], [1, w]])
            ohv = _ap(OHT, 0, [[K, C], [1, w]])
            rv_ = _ap(RT, 0, [[K, C], [1, w]])
            stj = _ap(ST, i + 1, [[K, C], [1, w]])
            vec._custom_dve(OW, out=owv, in0=x2j, in1=x1j,
                            s0=_ap(BX, 4 * i + 2, [[K * 4, C], [1, 1]]),
                            s1=_ap(BX, 4 * i + 0, [[K * 4, C], [1, 1]]))
            vec._custom_dve(OW, out=ohv, in0=y2j, in1=y1j,
                            s0=_ap(BX, 4 * i + 3, [[K * 4, C], [1, 1]]),
                            s1=_ap(BX, 4 * i + 1, [[K * 4, C], [1, 1]]))
            vec.tensor_mul(rv_, owv, ohv)
            vec._custom_dve(UPD, out=stj, in0=rv_, in1=stj,
                            s0=AL[:, i:i + 1], s1=KB[:])

        # ---- stage 10: outputs -------------------------------------------
        vec.tensor_scalar(ALV[:], ST[:], HALF_BIG, None, op0=ALU.is_lt)
        bx3 = _ap(BX, 0, [[K * 4, C], [4, K], [1, 4]])
        al3 = _ap(ALV, 0, [[K, C], [1, K], [0, 4]])
        vec.tensor_tensor(_ap(OUTT, 0, [[1000, C], [4, K], [1, 4]]), bx3, al3,
                          op=ALU.mult)
        vec.tensor_tensor(_ap(OUTT, 800, [[1000, C], [1, K]]), v200, ALV[:],
                          op=ALU.mult)
        nc.sync.dma_start(out.ap(), OUTT[:])

    nc.compile()
    _nc_cache["nc"] = nc
    return nc


def _shard_inputs(preds_loc, preds_conf, anchors):
    conf = np.asarray(preds_conf, np.float32).reshape(B, A, C)
    loc = np.ascontiguousarray(np.asarray(preds_loc, np.float32))
    anchors = np.ascontiguousarray(np.asarray(anchors, np.float32))
    in_maps = []
    for b in range(B):
        ct = np.zeros((CP, A), np.float32)
        ct[:C] = conf[b].T
        in_maps.append({
            "conf_t": np.ascontiguousarray(ct),
            "loc": loc[b],
            "anchors": anchors,
        })
    return in_maps


def kernel(preds_loc, preds_conf, anchors):
    nc = build_nc()
    in_maps = _shard_inputs(preds_loc, preds_conf, anchors)
    res = run_bass_kernel_spmd(nc, in_maps, core_ids=list(range(8)))
    det_loc = np.zeros((B, C, K, 4), np.float32)
    det_conf = np.zeros((B, C, K, 1), np.float32)
    for b in range(B):
        o = res.results[b]["out"].reshape(C, 1000)
        det_loc[b] = o[:, :800].reshape(C, K, 4)
        det_conf[b] = o[:, 800:].reshape(C, K, 1)
    return det_loc, det_conf
